# revision 1
# baseline (speedup 1.0000x reference)
"""CrystalGNN (GCNConv -> relu -> mean-pool -> FC -> log_softmax) on 8
Trainium2 NeuronCores.

Strategy (graph/data parallel, per sharding hint):
- 256 graphs -> 8 cores x 32 graphs. batch_idx is sorted, so each core owns a
  contiguous node range and every edge's *target* (col) lives on exactly one
  core. Edges are sharded by target.
- Each core computes the full normalized feature table
  h'[n] = dinv[n] * (x[n] @ W1) (redundant across cores; cheap dense matmul)
  into DRAM as bf16 rows padded to 256B, split into 4 chunks of 25088 rows so
  dma_gather's int16 indices can address any row.
- Message passing: per target-window of 128 local nodes, gather all source
  rows with dma_gather (one 256B descriptor per edge incl. self-loop), then
  segment-sum with one-hot T matmuls on the tensor engine accumulating in
  PSUM. Window results get dinv[col] scaling + b1 + relu.
- Mean-pool + bias folded into a matmul against a host-built B matrix
  (B[n,g] = 1[batch[n]==g]/cnt[g]); final FC + log_softmax on device.

Self-contained: only needs numpy/ml_dtypes + the concourse stack at
/opt/trn_rl_repo (or already on sys.path).
"""
import sys

for _p in ("/opt/trn_rl_repo",):
    if _p not in sys.path:
        sys.path.append(_p)

import numpy as np
import ml_dtypes

import concourse.bass as bass
import concourse.bacc as bacc
import concourse.mybir as mybir
import concourse.tile as tile
from concourse import bass_utils

P = 128
NCORES = 8
NGRAPH = 256
GPC = NGRAPH // NCORES        # graphs per core
N = 100000                    # nodes
FIN = 128                     # input features
H = 64                        # hidden
CHUNK = 25088                 # h' table chunk rows (196*128, int16-addressable)
NCHUNK = 4
NTAB = CHUNK * NCHUNK         # 100352 padded table rows
SW = 4                        # windows per superwindow (PSUM banks)
HSTRIP = 14                   # h-phase blocks per DMA strip (196 = 14*14)
CALLCAP = 8                   # max gather blocks (x128 idx) per dma_gather call

bf16 = ml_dtypes.bfloat16


# ----------------------------------------------------------------- schedule
def build_schedule(x, edge_index, batch_idx, W1, b1, Wfc, bfc):
    """Host-side preprocessing: sharding, slot schedule, index/one-hot data."""
    x = np.asarray(x)
    edge_index = np.asarray(edge_index).astype(np.int64)
    batch = np.asarray(batch_idx).astype(np.int64)
    W1 = np.asarray(W1, dtype=np.float32)
    b1 = np.asarray(b1, dtype=np.float32)
    Wfc = np.asarray(Wfc, dtype=np.float32)
    bfc = np.asarray(bfc, dtype=np.float32)

    row, col = edge_index[0], edge_index[1]
    deg = np.bincount(col, minlength=N).astype(np.float32) + 1.0
    dinv = (1.0 / np.sqrt(deg)).astype(np.float32)

    gcnt = np.bincount(batch, minlength=NGRAPH).astype(np.float32)
    assert (gcnt > 0).all(), "empty graphs unsupported"
    inv_cnt = 1.0 / gcnt

    starts = np.searchsorted(batch, np.arange(NCORES) * GPC, side="left")
    ends = np.searchsorted(batch, (np.arange(NCORES) + 1) * GPC, side="left")
    nk = ends - starts
    NWIN = int(np.ceil(nk.max() / P))
    NSW = (NWIN + SW - 1) // SW

    ecore = np.searchsorted(ends, col, side="right")  # core of each edge (by target)

    # per-core edge arrays (target-sharded) with self-loops appended
    core_src, core_w, core_c, core_cl = [], [], [], []
    for k in range(NCORES):
        m = ecore == k
        r_k = row[m]
        c_k = col[m]
        nloc = np.arange(starts[k], ends[k], dtype=np.int64)
        src = np.concatenate([r_k, nloc])
        tgt = np.concatenate([c_k, nloc]) - starts[k]
        core_src.append(src)
        core_w.append(tgt >> 7)
        core_cl.append(tgt & 127)
        core_c.append(src // CHUNK)

    # shared block schedule: B[w, c] = ceil(max_k slots_k(w,c) / 128), >= 1
    Btab = np.ones((NWIN, NCHUNK), dtype=np.int64)
    for k in range(NCORES):
        key = core_w[k] * NCHUNK + core_c[k]
        cnt = np.bincount(key, minlength=NWIN * NCHUNK).reshape(NWIN, NCHUNK)
        Btab = np.maximum(Btab, (cnt + P - 1) // P)

    # emission order: for sw, for chunk, for w in sw, blocks
    blk_w, blk_c = [], []
    call_cb = []          # blocks per call (sw, c)
    blk_base = np.zeros((NWIN, NCHUNK), dtype=np.int64)
    K = 0
    for s in range(NSW):
        ws = range(s * SW, min((s + 1) * SW, NWIN))
        for c in range(NCHUNK):
            cb = 0
            for w in ws:
                blk_base[w, c] = K
                b = int(Btab[w, c])
                blk_w += [w] * b
                blk_c += [c] * b
                K += b
                cb += b
            call_cb.append(cb)
    TOTBLK = K
    TOTSLOT = TOTBLK * P

    # per-core slot data
    idx16 = np.zeros((NCORES, P, TOTSLOT // 16), dtype=np.int16)
    colv = np.full((NCORES, P, TOTBLK), -1.0, dtype=np.float32)
    for k in range(NCORES):
        key = core_w[k] * NCHUNK + core_c[k]
        order = np.argsort(key, kind="stable")
        skey = key[order]
        ssrc = core_src[k][order]
        scl = core_cl[k][order]
        seg_start = np.searchsorted(skey, np.arange(NWIN * NCHUNK), side="left")
        seg_end = np.searchsorted(skey, np.arange(NWIN * NCHUNK), side="right")
        gidx = np.zeros(TOTSLOT, dtype=np.int16)
        for w in range(NWIN):
            for c in range(NCHUNK):
                a, b = seg_start[w * NCHUNK + c], seg_end[w * NCHUNK + c]
                n = b - a
                if n == 0:
                    continue
                base = blk_base[w, c] * P
                pos = base + np.arange(n)
                gidx[pos] = (ssrc[a:b] - c * CHUNK).astype(np.int16)
                colv[k, pos & 127, pos >> 7] = scl[a:b].astype(np.float32)
        # wrap per call: within call, wrapped[q, s] = idx[s*16+q]
        off = 0
        off16 = 0
        for cb in call_cb:
            nslot = cb * P
            wr = gidx[off:off + nslot].reshape(-1, 16).T  # [16, nslot//16]
            idx16[k, :, off16:off16 + nslot // 16] = np.tile(wr, (8, 1))
            off += nslot
            off16 += nslot // 16

    # B pooling matrix and dinv per window
    Bmat = np.zeros((NCORES, P, NWIN * GPC), dtype=bf16)
    dinv_win = np.zeros((NCORES, P, NWIN), dtype=np.float32)
    for k in range(NCORES):
        nn = int(nk[k])
        nodes = np.arange(starts[k], ends[k])
        g = batch[nodes] - k * GPC
        w = np.arange(nn) >> 7
        p = np.arange(nn) & 127
        Bm = np.zeros((P, NWIN, GPC), dtype=np.float32)
        Bm[p, w, g] = inv_cnt[batch[nodes]]
        Bmat[k] = Bm.reshape(P, NWIN * GPC).astype(bf16)
        dv = np.zeros((P, NWIN), dtype=np.float32)
        dv[p, w] = dinv[nodes]
        dinv_win[k] = dv

    # shared tensors
    xT = np.zeros((FIN, NTAB), dtype=bf16)
    xT[:, :N] = np.asarray(x, dtype=np.float32).T.astype(bf16)
    dinv_pad = np.zeros(NTAB, dtype=np.float32)
    dinv_pad[:N] = dinv
    dinv_blk = np.ascontiguousarray(dinv_pad.reshape(NTAB // P, P).T)  # [p, b] = dinv[128*b+p]
    b1b = np.broadcast_to(b1, (P, H)).astype(np.float32).copy()
    wfce = np.concatenate([Wfc, bfc[None, :]], axis=0).astype(np.float32)  # [65, 2]
    iota = np.broadcast_to(np.arange(P, dtype=np.float32), (P, P)).copy()
    ident = np.eye(P, dtype=np.float32)

    return dict(
        NWIN=NWIN, NSW=NSW, Btab=Btab, call_cb=call_cb, TOTBLK=TOTBLK,
        idx16=idx16, colv=colv, Bmat=Bmat, dinv_win=dinv_win,
        xT=xT, dinv_blk=dinv_blk, b1b=b1b, wfce=wfce, iota=iota, ident=ident,
        W1=W1.astype(bf16),
    )


# ------------------------------------------------------------------ kernel IR
def build_nc(sched, num_devices=NCORES):
    NWIN, NSW = sched["NWIN"], sched["NSW"]
    Btab, call_cb, TOTBLK = sched["Btab"], sched["call_cb"], sched["TOTBLK"]
    f32, bft, i16 = mybir.dt.float32, mybir.dt.bfloat16, mybir.dt.int16

    nc = bacc.Bacc("TRN2", target_bir_lowering=False, debug=False,
                   num_devices=num_devices)
    d_xT = nc.dram_tensor("xT", [FIN, NTAB], bft, kind="ExternalInput")
    d_W1 = nc.dram_tensor("W1", [FIN, H], bft, kind="ExternalInput")
    d_dblk = nc.dram_tensor("dinv_blk", [P, NTAB // P], f32, kind="ExternalInput")
    d_idx = nc.dram_tensor("idx16", [P, TOTBLK * 8], i16, kind="ExternalInput")
    d_colv = nc.dram_tensor("colv", [P, TOTBLK], f32, kind="ExternalInput")
    d_B = nc.dram_tensor("Bmat", [P, NWIN * GPC], bft, kind="ExternalInput")
    d_dwin = nc.dram_tensor("dinv_win", [P, NWIN], f32, kind="ExternalInput")
    d_b1b = nc.dram_tensor("b1b", [P, H], f32, kind="ExternalInput")
    d_wfce = nc.dram_tensor("wfce", [H + 1, 2], f32, kind="ExternalInput")
    d_iota = nc.dram_tensor("iota", [P, P], f32, kind="ExternalInput")
    d_ident = nc.dram_tensor("ident", [P, P], f32, kind="ExternalInput")
    d_out = nc.dram_tensor("outd", [GPC, 2], f32, kind="ExternalOutput")

    NBLK_H = CHUNK // P  # 196 h-blocks per chunk

    with tile.TileContext(nc) as tc:
        with tc.tile_pool(name="const", bufs=1) as cp, \
             tc.tile_pool(name="hio", bufs=3) as hio, \
             tc.tile_pool(name="gio", bufs=3) as gio, \
             tc.tile_pool(name="tp", bufs=4) as tpool, \
             tc.tile_pool(name="wio", bufs=3) as wio, \
             tc.tile_pool(name="hps", bufs=2, space="PSUM") as hps, \
             tc.tile_pool(name="aggps", bufs=SW, space="PSUM") as aggps, \
             tc.tile_pool(name="poolps", bufs=1, space="PSUM") as poolps, \
             tc.tile_pool(name="dram", bufs=1, space="DRAM") as dp:

            # constants
            w1_t = cp.tile([FIN, H], bft, tag="w1")
            nc.sync.dma_start(out=w1_t[:], in_=d_W1[:])
            dblk_t = cp.tile([P, NTAB // P], f32, tag="dblk")
            nc.sync.dma_start(out=dblk_t[:], in_=d_dblk[:])
            iota_t = cp.tile([P, P], f32, tag="iota")
            nc.sync.dma_start(out=iota_t[:], in_=d_iota[:])
            ident_t = cp.tile([P, P], f32, tag="ident")
            nc.sync.dma_start(out=ident_t[:], in_=d_ident[:])
            b1b_t = cp.tile([P, H], f32, tag="b1b")
            nc.sync.dma_start(out=b1b_t[:], in_=d_b1b[:])
            wfce_t = cp.tile([H + 1, 2], f32, tag="wfce")
            nc.sync.dma_start(out=wfce_t[:], in_=d_wfce[:])
            idx_t = cp.tile([P, TOTBLK * 8], i16, tag="idx")
            nc.sync.dma_start(out=idx_t[:], in_=d_idx[:])
            colv_t = cp.tile([P, TOTBLK], f32, tag="colv")
            nc.sync.dma_start(out=colv_t[:], in_=d_colv[:])
            bmat_t = cp.tile([P, NWIN * GPC], bft, tag="bmat")
            nc.sync.dma_start(out=bmat_t[:], in_=d_B[:])
            dwin_t = cp.tile([P, NWIN], f32, tag="dwin")
            nc.sync.dma_start(out=dwin_t[:], in_=d_dwin[:])
            ones_t = cp.tile([P, 1], bft, tag="ones")
            nc.vector.memset(ones_t[:], 1.0)

            # h' chunk tables in DRAM (bf16 rows padded to 256B: [r, 128])
            hbuf = [dp.tile([CHUNK, P], bft, tag=f"hbuf{c}", name=f"hbuf{c}")
                    for c in range(NCHUNK)]

            # ---- phase 1: h' = dinv * (x @ W1), streamed per chunk ----
            for c in range(NCHUNK):
                for s in range(NBLK_H // HSTRIP):
                    b0 = c * NBLK_H + s * HSTRIP  # global block
                    xs = hio.tile([P, HSTRIP * P], bft, tag="xs")
                    nc.sync.dma_start(
                        out=xs[:], in_=d_xT[:, b0 * P:(b0 + HSTRIP) * P])
                    hst = hio.tile([P, HSTRIP, H], bft, tag="hst")
                    for j in range(HSTRIP):
                        hp = hps.tile([P, H], f32, tag="hp")
                        nc.tensor.matmul(
                            out=hp[:], lhsT=xs[:, j * P:(j + 1) * P],
                            rhs=w1_t[:], start=True, stop=True)
                        nc.vector.tensor_scalar(
                            out=hst[:, j, :], in0=hp[:],
                            scalar1=dblk_t[:, b0 + j:b0 + j + 1], scalar2=None,
                            op0=mybir.AluOpType.mult)
                    # store rows [s*HSTRIP*P, ...) of chunk c (cols 0:64)
                    dst = hbuf[c][s * HSTRIP * P:(s + 1) * HSTRIP * P, 0:H]
                    nc.sync.dma_start(
                        out=dst.rearrange("(j p) h -> p j h", p=P), in_=hst[:])

            # ---- phase 2: gather + segment-sum + pool ----
            pool_ps = poolps.tile([H + 1, GPC], f32, tag="pool")
            blk = 0      # global block counter
            off16 = 0    # idx16 column offset
            nwin_done = 0
            for s in range(NSW):
                ws = list(range(s * SW, min((s + 1) * SW, NWIN)))
                agg = {w: aggps.tile([P, H], f32, tag="agg", name=f"agg{w}")
                       for w in ws}
                for c in range(NCHUNK):
                    # ordered blocks of this (sw, chunk) call group
                    blist = []
                    for w in ws:
                        nb = int(Btab[w, c])
                        for b in range(nb):
                            blist.append((w, c == 0 and b == 0,
                                          c == NCHUNK - 1 and b == nb - 1))
                    # gather in sub-calls of <= CALLCAP blocks (SWDGE ring cap)
                    for g0 in range(0, len(blist), CALLCAP):
                        grp = blist[g0:g0 + CALLCAP]
                        cb = len(grp)
                        msg = gio.tile([P, CALLCAP, P], bft, tag="msg")
                        nc.gpsimd.dma_gather(
                            out_ap=msg[:, 0:cb, :], in_ap=hbuf[c][:],
                            idxs_ap=idx_t[:, off16:off16 + cb * 8],
                            num_idxs=cb * P, num_idxs_reg=cb * P, elem_size=P)
                        off16 += cb * 8
                        for bi, (w, first, last) in enumerate(grp):
                            T = tpool.tile([P, P], bft, tag="T")
                            nc.vector.tensor_scalar(
                                out=T[:], in0=iota_t[:],
                                scalar1=colv_t[:, blk:blk + 1], scalar2=None,
                                op0=mybir.AluOpType.is_equal)
                            nc.tensor.matmul(
                                out=agg[w][:], lhsT=T[:],
                                rhs=msg[:, bi, 0:H],
                                start=first, stop=last)
                            blk += 1
                # window epilogue: scale, bias, relu, pool
                for w in ws:
                    sc = wio.tile([P, H], f32, tag="sc")
                    nc.vector.tensor_scalar(
                        out=sc[:], in0=agg[w][:], scalar1=dwin_t[:, w:w + 1],
                        scalar2=None, op0=mybir.AluOpType.mult)
                    sb = wio.tile([P, H], f32, tag="sb")
                    nc.vector.tensor_tensor(
                        out=sb[:], in0=sc[:], in1=b1b_t[:],
                        op=mybir.AluOpType.add)
                    rl = wio.tile([P, H], bft, tag="rl")
                    nc.scalar.activation(
                        out=rl[:], in_=sb[:],
                        func=mybir.ActivationFunctionType.Relu)
                    first = nwin_done == 0
                    last = nwin_done == NWIN - 1
                    nc.tensor.matmul(
                        out=pool_ps[0:H, :], lhsT=rl[:],
                        rhs=bmat_t[:, w * GPC:(w + 1) * GPC],
                        start=first, stop=last, skip_group_check=True)
                    nc.tensor.matmul(
                        out=pool_ps[H:H + 1, :], lhsT=ones_t[:],
                        rhs=bmat_t[:, w * GPC:(w + 1) * GPC],
                        start=first, stop=last, skip_group_check=True)
                    nwin_done += 1

            # ---- phase 3: FC + log_softmax ----
            plc = cp.tile([H + 1, GPC], f32, tag="plc")
            nc.vector.tensor_copy(out=plc[:], in_=pool_ps[:])
            lg_ps = hps.tile([2, GPC], f32, tag="hp")
            nc.tensor.matmul(out=lg_ps[:], lhsT=wfce_t[:], rhs=plc[:],
                             start=True, stop=True)
            lgs = cp.tile([2, GPC], f32, tag="lgs")
            nc.vector.tensor_copy(out=lgs[:], in_=lg_ps[:])
            tr_ps = hps.tile([GPC, 2], f32, tag="hp")
            nc.tensor.transpose(out=tr_ps[:], in_=lgs[:], identity=ident_t[:2, :2])
            ls = cp.tile([GPC, 2], f32, tag="ls")
            nc.vector.tensor_copy(out=ls[:], in_=tr_ps[:])
            nm = cp.tile([GPC, 1], f32, tag="nm")
            nc.vector.tensor_reduce(out=nm[:], in_=ls[:],
                                    axis=mybir.AxisListType.X,
                                    op=mybir.AluOpType.max, negate=True)
            ex = cp.tile([GPC, 2], f32, tag="ex")
            nc.scalar.activation(out=ex[:], in_=ls[:],
                                 func=mybir.ActivationFunctionType.Exp,
                                 bias=nm[:, 0:1])
            ssum = cp.tile([GPC, 1], f32, tag="ssum")
            nc.vector.tensor_reduce(out=ssum[:], in_=ex[:],
                                    axis=mybir.AxisListType.X,
                                    op=mybir.AluOpType.add)
            lse = cp.tile([GPC, 1], f32, tag="lse")
            nc.scalar.activation(out=lse[:], in_=ssum[:],
                                 func=mybir.ActivationFunctionType.Ln)
            fin = cp.tile([GPC, 2], f32, tag="fin")
            nc.vector.tensor_scalar(
                out=fin[:], in0=ls[:], scalar1=nm[:, 0:1], scalar2=lse[:, 0:1],
                op0=mybir.AluOpType.add, op1=mybir.AluOpType.subtract)
            nc.sync.dma_start(out=d_out[:], in_=fin[:])

    nc.compile()
    return nc


def make_in_maps(sched):
    maps = []
    for k in range(NCORES):
        maps.append({
            "xT": sched["xT"], "W1": sched["W1"],
            "dinv_blk": sched["dinv_blk"],
            "idx16": sched["idx16"][k], "colv": sched["colv"][k],
            "Bmat": sched["Bmat"][k], "dinv_win": sched["dinv_win"][k],
            "b1b": sched["b1b"], "wfce": sched["wfce"],
            "iota": sched["iota"], "ident": sched["ident"],
        })
    return maps


def kernel(**inputs) -> np.ndarray:
    sched = build_schedule(**inputs)
    nc = build_nc(sched)
    res = bass_utils.run_bass_kernel_spmd(
        nc, make_in_maps(sched), core_ids=list(range(NCORES)))
    out = np.concatenate([res.results[k]["outd"] for k in range(NCORES)], axis=0)
    return out.astype(np.float32)

